# revision 12
# baseline (speedup 1.0000x reference)
"""Trainium2 Bass kernel for nn_BasicRNN (8-core SPMD).

Problem: 2048-step LayerNorm RNN over [B=32, E=512] + embedding gather +
vocab projection with log_softmax (V=32000).

Math reformulation (the key to a cheap serial step):
  reference step:  z' = LN(u @ A + b_dec),  u = z + x_t,  A = I + W_dec
  with LN(v) = gamma*(v - mu)/(std1(v) + eps) + beta  (unbiased std).
  Writing z = gamma*n + beta and folding constants:
    h_{t+1} = r_t * (h_t @ Atg) + xt_{t+1}
  where Atg = (I - 11^T/E) @ diag(gamma) @ A   (centering absorbed),
        r_t = 1/(std1(h_t) + eps),
        xt_t = x_t @ A + (beta @ A + b_dec)    (precomputed in batch phase),
        h_0  = xt_0 + (b_dec - c).
  Final: z = gamma*(h_S - mu)/(std1+eps) + beta;  y = log_softmax(z@Wv + bv).

Distribution (8 cores):
  - batch phase (embedding gather + x@A) data-parallel over interleaved
    32-step time blocks; exchanged via 8 chunked AllGathers so the scan
    starts after ~1/8 of the exchange.
  - the serial scan is replicated on every core (it is latency-bound and
    B<=128 fits one matmul; replication removes all syncs).
  - vocab projection is tensor-parallel over V (4000 cols/core); one tiny
    AllGather of per-core (max, sumexp) stats completes log_softmax.
"""
import sys

sys.path.insert(0, "/opt/trn_rl_repo")

import numpy as np
from contextlib import ExitStack

import concourse.bass as bass
import concourse.bacc as bacc
import concourse.tile as tile
from concourse import mybir
from concourse.bass_utils import run_bass_kernel_spmd
from concourse.masks import make_identity

f32 = mybir.dt.float32
i32 = mybir.dt.int32
AF = mybir.ActivationFunctionType
OP = mybir.AluOpType

B, S, E, V = 32, 2048, 512, 32000
NCORES = 8
VS = V // NCORES          # vocab shard per core
EC = E // 128             # 4 e-chunks
ROWS = (S // NCORES) * B  # 8192 gathered rows per core
NTILES = ROWS // 128      # 64 gather tiles of 128 rows
NCHUNK = 8                # chunked allgathers of the xt exchange
TPC = NTILES // NCHUNK    # 8 gather tiles per chunk
EPS = 1e-6
VAR_CORR = float(E) / float(E - 1)  # ddof=1 correction inside sqrt

PEEL = 256                # steps done statically (waits only on chunk 0)
GROUP = 16                # steps per xt DMA group
LOOP_BODY = 16            # steps per For_i iteration
N_ITERS = (S - PEEL) // LOOP_BODY
assert PEEL + N_ITERS * LOOP_BODY == S

_CACHE = {}


def _emit_xt_group(nc, pool, xt_full, row0, nsteps):
    """DMA xt rows [row0, row0+32*nsteps) into an SBUF tile [B, nsteps, E]."""
    t = pool.tile([B, nsteps, E], f32, tag="xtg")
    src = xt_full[bass.ds(row0, 32 * nsteps), :].rearrange("(s b) e -> b s e", b=B)
    nc.sync.dma_start(out=t, in_=src)
    return t


def _emit_step(nc, ctx, st, xt_tile, s):
    """One scan step. On entry: st.h = h_t (natural [B,E]), st.hT = h_t^T.
    Consumes xt slice s. On exit: st.h/st.hT hold h_{t+1}."""
    # stats of h_t -> r_t (DVE/ACT, overlaps the matmul below)
    st6 = st.stat.tile([B, 6], f32, tag="st6")
    mv = st.stat.tile([B, 2], f32, tag="mv")
    nc.vector.bn_stats(out=st6, in_=st.h[:, :])
    nc.vector.bn_aggr(out=mv, in_=st6)
    stde = st.stat.tile([B, 1], f32, tag="stde")
    nc.scalar.activation(out=stde, in_=mv[:, 1:2], func=AF.Sqrt, scale=VAR_CORR)
    nc.vector.tensor_scalar_add(stde, stde, EPS)
    r = st.stat.tile([B, 1], f32, tag="r")
    nc.vector.reciprocal(out=r, in_=stde)

    # p = h_t @ Atg  (PE, stationary = hT chunks)
    p = st.psum.tile([B, E], f32, tag="pps")
    for k in range(EC):
        nc.tensor.matmul(
            p,
            lhsT=st.hT[:, k, :],
            rhs=st.atg[:, k, :],
            start=(k == 0),
            stop=(k == EC - 1),
        )

    # h_{t+1} = r * p + xt   (chunked so transposes can start early)
    for k in range(EC):
        sl = slice(k * 128, (k + 1) * 128)
        nc.vector.scalar_tensor_tensor(
            out=st.h[:, sl],
            in0=p[:, sl],
            scalar=r,
            in1=xt_tile[:, s, sl],
            op0=OP.mult,
            op1=OP.add,
        )
    # transpose h_{t+1} -> hT  (PE transpose + ACT copy out of PSUM)
    for k in range(EC):
        sl = slice(k * 128, (k + 1) * 128)
        tp = st.tpsum.tile([128, B], f32, tag="tps")
        nc.tensor.transpose(out=tp, in_=st.h[:, sl], identity=st.ident32)
        nc.scalar.copy(out=st.hT[:, k, :], in_=tp)


class _St:
    pass


def _build():
    nc = bacc.Bacc(num_devices=NCORES)

    emb = nc.declare_dram_parameter("emb", [V, E], f32, isOutput=False)
    gidx = nc.declare_dram_parameter("gidx", [128, NTILES], i32, isOutput=False)
    atg_d = nc.declare_dram_parameter("atg", [E, E], f32, isOutput=False)
    abat_d = nc.declare_dram_parameter("abat", [E, E], f32, isOutput=False)
    cvec = nc.declare_dram_parameter("cvec", [1, E], f32, isOutput=False)
    corr0 = nc.declare_dram_parameter("corr0", [1, E], f32, isOutput=False)
    gam = nc.declare_dram_parameter("gam", [1, E], f32, isOutput=False)
    bet = nc.declare_dram_parameter("bet", [1, E], f32, isOutput=False)
    wvoc = nc.declare_dram_parameter("wvoc", [E, VS], f32, isOutput=False)
    bvoc = nc.declare_dram_parameter("bvoc", [1, VS], f32, isOutput=False)
    z_out = nc.declare_dram_parameter("z_out", [B, E], f32, isOutput=True)
    y_out = nc.declare_dram_parameter("y_out", [B, VS], f32, isOutput=True)

    xt_loc = nc.dram_tensor("xt_loc", [ROWS, E], f32)
    xt_full = nc.dram_tensor("xt_full", [S * B, E], f32, addr_space="Shared")
    stats_in = nc.dram_tensor("stats_in", [B, 2], f32)
    stats_out = nc.dram_tensor("stats_out", [NCORES * B, 2], f32, addr_space="Shared")

    def bcast(ap, parts):
        return bass.AP(tensor=ap.tensor, offset=ap.offset, ap=[[0, parts]] + ap.ap[1:])

    with tile.TileContext(nc) as tc, ExitStack() as octx:
        consts = octx.enter_context(tc.tile_pool(name="consts", bufs=1))
        state = octx.enter_context(tc.tile_pool(name="state", bufs=1))
        statp = octx.enter_context(tc.tile_pool(name="stats", bufs=2))
        xpool = octx.enter_context(tc.tile_pool(name="xt", bufs=2))
        ppool = octx.enter_context(tc.tile_pool(name="psum", bufs=2, space="PSUM"))
        tppool = octx.enter_context(tc.tile_pool(name="tpsum", bufs=2, space="PSUM"))

        # ---- scan-lifetime constants ----
        ident32 = consts.tile([32, 32], f32)
        make_identity(nc, ident32)
        atg_sb = consts.tile([128, EC, E], f32)
        nc.sync.dma_start(out=atg_sb, in_=atg_d[:].rearrange("(k p) n -> p k n", p=128))
        wv_sb = consts.tile([128, EC, VS], f32)
        nc.sync.dma_start(out=wv_sb, in_=wvoc[:].rearrange("(k p) n -> p k n", p=128))
        corr0_bc = consts.tile([B, E], f32)
        nc.sync.dma_start(out=corr0_bc, in_=bcast(corr0[:], B))

        # ---- batch phase: gather + x@A (+c), write xt_loc, chunked AllGather
        with ExitStack() as bctx:
            gp = bctx.enter_context(tc.tile_pool(name="bgather", bufs=3))
            bxt = bctx.enter_context(tc.tile_pool(name="bxT", bufs=2))
            bps = bctx.enter_context(tc.tile_pool(name="bpsum", bufs=2, space="PSUM"))
            bmm = bctx.enter_context(tc.tile_pool(name="bmm", bufs=2, space="PSUM"))
            bout = bctx.enter_context(tc.tile_pool(name="bout", bufs=3))
            bconst = bctx.enter_context(tc.tile_pool(name="bconst", bufs=1))

            ident128 = bconst.tile([128, 128], f32)
            make_identity(nc, ident128)
            abat_sb = bconst.tile([128, EC, E], f32)
            nc.sync.dma_start(
                out=abat_sb, in_=abat_d[:].rearrange("(k p) n -> p k n", p=128)
            )
            c_bc = bconst.tile([128, E], f32)
            nc.sync.dma_start(out=c_bc, in_=bcast(cvec[:], 128))
            idx_sb = bconst.tile([128, NTILES], i32)
            nc.sync.dma_start(out=idx_sb, in_=gidx[:])

            for j in range(NTILES):
                xg = gp.tile([128, E], f32, tag="xg")
                nc.gpsimd.indirect_dma_start(
                    out=xg[:],
                    out_offset=None,
                    in_=emb[:],
                    in_offset=bass.IndirectOffsetOnAxis(
                        ap=idx_sb[:, j : j + 1], axis=0
                    ),
                )
                xT = bxt.tile([128, EC, 128], f32, tag="xT")
                for k in range(EC):
                    tp = bps.tile([128, 128], f32, tag="btp")
                    nc.tensor.transpose(
                        out=tp, in_=xg[:, k * 128 : (k + 1) * 128], identity=ident128
                    )
                    nc.vector.tensor_copy(out=xT[:, k, :], in_=tp)
                ps = bmm.tile([128, E], f32, tag="bps")
                for k in range(EC):
                    nc.tensor.matmul(
                        ps,
                        lhsT=xT[:, k, :],
                        rhs=abat_sb[:, k, :],
                        start=(k == 0),
                        stop=(k == EC - 1),
                    )
                xts = bout.tile([128, E], f32, tag="xts")
                nc.vector.tensor_add(out=xts, in0=ps, in1=c_bc)
                nc.sync.dma_start(out=xt_loc[j * 128 : (j + 1) * 128, :], in_=xts)

                if (j + 1) % TPC == 0:
                    jj = j // TPC
                    nc.gpsimd.collective_compute(
                        "AllGather",
                        OP.bypass,
                        replica_groups=[list(range(NCORES))],
                        ins=[xt_loc[jj * TPC * 128 : (jj + 1) * TPC * 128, :]],
                        outs=[xt_full[jj * NCORES * TPC * 128 : (jj + 1) * NCORES * TPC * 128, :]],
                    )

        # ---- scan ----
        st = _St()
        st.h = state.tile([B, E], f32)
        st.hT = state.tile([128, EC, B], f32)
        st.atg = atg_sb
        st.ident32 = ident32
        st.stat = statp
        st.psum = ppool
        st.tpsum = tppool

        # init: h_0 = xt_0 + corr0, then hT_0
        g0 = _emit_xt_group(nc, xpool, xt_full, 0, GROUP)
        nc.vector.tensor_add(out=st.h[:, :], in0=g0[:, 0, :], in1=corr0_bc[:, :])
        for k in range(EC):
            sl = slice(k * 128, (k + 1) * 128)
            tp0 = st.tpsum.tile([128, B], f32, tag="tps")
            nc.tensor.transpose(out=tp0, in_=st.h[:, sl], identity=ident32)
            nc.scalar.copy(out=st.hT[:, k, :], in_=tp0)

        # peeled static steps t = 1..PEEL-1
        for t in range(1, GROUP):
            _emit_step(nc, octx, st, g0, t)
        for t0 in range(GROUP, PEEL, GROUP):
            g = _emit_xt_group(nc, xpool, xt_full, 32 * t0, GROUP)
            for s in range(GROUP):
                _emit_step(nc, octx, st, g, s)

        # main loop t = PEEL..S-1
        with tc.For_i(32 * PEEL, 32 * S, 32 * LOOP_BODY) as v:
            g = _emit_xt_group(nc, xpool, xt_full, v, LOOP_BODY)
            for s in range(LOOP_BODY):
                _emit_step(nc, octx, st, g, s)

        # ---- tail: final LN apply, projection, log_softmax ----
        with ExitStack() as tctx:
            tpool = tctx.enter_context(tc.tile_pool(name="tail", bufs=1))
            tps8 = tctx.enter_context(tc.tile_pool(name="tailps", bufs=4, space="PSUM"))

            gam_bc = tpool.tile([B, E], f32)
            nc.sync.dma_start(out=gam_bc, in_=bcast(gam[:], B))
            bet_bc = tpool.tile([B, E], f32)
            nc.sync.dma_start(out=bet_bc, in_=bcast(bet[:], B))
            bvoc_bc = tpool.tile([B, VS], f32)
            nc.sync.dma_start(out=bvoc_bc, in_=bcast(bvoc[:], B))

            st6 = tpool.tile([B, 6], f32)
            mv = tpool.tile([B, 2], f32)
            nc.vector.bn_stats(out=st6, in_=st.h[:, :])
            nc.vector.bn_aggr(out=mv, in_=st6)
            stde = tpool.tile([B, 1], f32)
            nc.scalar.activation(out=stde, in_=mv[:, 1:2], func=AF.Sqrt, scale=VAR_CORR)
            nc.vector.tensor_scalar_add(stde, stde, EPS)
            r = tpool.tile([B, 1], f32)
            nc.vector.reciprocal(out=r, in_=stde)
            zt = tpool.tile([B, E], f32)
            nc.vector.tensor_scalar(
                out=zt, in0=st.h[:, :], scalar1=mv[:, 0:1], scalar2=r,
                op0=OP.subtract, op1=OP.mult,
            )
            z_sb = tpool.tile([B, E], f32)
            nc.vector.tensor_tensor(out=z_sb, in0=zt, in1=gam_bc, op=OP.mult)
            nc.vector.tensor_tensor(out=z_sb, in0=z_sb, in1=bet_bc, op=OP.add)
            nc.sync.dma_start(out=z_out[:], in_=z_sb)

            # zT for projection
            zT = tpool.tile([128, EC, B], f32)
            for k in range(EC):
                tp = st.tpsum.tile([128, B], f32, tag="tps")
                nc.tensor.transpose(
                    out=tp, in_=z_sb[:, k * 128 : (k + 1) * 128], identity=ident32
                )
                nc.scalar.copy(out=zT[:, k, :], in_=tp)

            # logits = z @ Wv + bv  (8 n-tiles of 500)
            NT = 8
            NW = VS // NT
            logits = tpool.tile([B, VS], f32)
            for n in range(NT):
                lp = tps8.tile([B, NW], f32, tag="lg")
                for k in range(EC):
                    nc.tensor.matmul(
                        lp,
                        lhsT=zT[:, k, :],
                        rhs=wv_sb[:, k, n * NW : (n + 1) * NW],
                        start=(k == 0),
                        stop=(k == EC - 1),
                    )
                nc.vector.tensor_add(
                    out=logits[:, n * NW : (n + 1) * NW],
                    in0=lp,
                    in1=bvoc_bc[:, n * NW : (n + 1) * NW],
                )

            mx = tpool.tile([B, 1], f32)
            nc.vector.tensor_reduce(
                out=mx, in_=logits, axis=mybir.AxisListType.X, op=OP.max
            )
            negmx = tpool.tile([B, 1], f32)
            nc.scalar.mul(out=negmx, in_=mx, mul=-1.0)
            # per-n-tile exp with accumulation (avoids a [B, VS] scratch)
            scr = tpool.tile([B, NW], f32)
            parts = tpool.tile([B, NT], f32)
            for n in range(NT):
                nc.scalar.activation(
                    out=scr,
                    in_=logits[:, n * NW : (n + 1) * NW],
                    func=AF.Exp,
                    bias=negmx,
                    accum_out=parts[:, n : n + 1],
                )
            sume = tpool.tile([B, 1], f32)
            nc.vector.tensor_reduce(
                out=sume, in_=parts, axis=mybir.AxisListType.X, op=OP.add
            )
            stats_sb = tpool.tile([B, 2], f32)
            nc.vector.tensor_copy(out=stats_sb[:, 0:1], in_=mx)
            nc.vector.tensor_copy(out=stats_sb[:, 1:2], in_=sume)
            nc.sync.dma_start(out=stats_in[:], in_=stats_sb)
            nc.gpsimd.collective_compute(
                "AllGather",
                OP.bypass,
                replica_groups=[list(range(NCORES))],
                ins=[stats_in[:]],
                outs=[stats_out[:]],
            )
            m8 = tpool.tile([B, NCORES], f32)
            s8 = tpool.tile([B, NCORES], f32)
            gview = stats_out[:].rearrange("(c b) f -> b c f", c=NCORES)
            nc.sync.dma_start(out=m8, in_=gview[:, :, 0:1])
            nc.sync.dma_start(out=s8, in_=gview[:, :, 1:2])
            gmax = tpool.tile([B, 1], f32)
            nc.vector.tensor_reduce(out=gmax, in_=m8, axis=mybir.AxisListType.X, op=OP.max)
            neggm = tpool.tile([B, 1], f32)
            nc.scalar.mul(out=neggm, in_=gmax, mul=-1.0)
            ex8 = tpool.tile([B, NCORES], f32)
            nc.scalar.activation(out=ex8, in_=m8, func=AF.Exp, bias=neggm)
            contrib = tpool.tile([B, NCORES], f32)
            nc.vector.tensor_tensor(out=contrib, in0=ex8, in1=s8, op=OP.mult)
            sg = tpool.tile([B, 1], f32)
            nc.vector.tensor_reduce(out=sg, in_=contrib, axis=mybir.AxisListType.X, op=OP.add)
            lse = tpool.tile([B, 1], f32)
            nc.scalar.activation(out=lse, in_=sg, func=AF.Ln)
            d = tpool.tile([B, 1], f32)
            nc.vector.tensor_tensor(out=d, in0=lse, in1=gmax, op=OP.add)
            negd = tpool.tile([B, 1], f32)
            nc.scalar.mul(out=negd, in_=d, mul=-1.0)
            nc.vector.tensor_scalar(
                out=logits, in0=logits, scalar1=negd, scalar2=None,
                op0=OP.add, op1=OP.bypass,
            )
            nc.sync.dma_start(out=y_out[:], in_=logits)

    nc.compile()
    return nc


def _get_nc():
    if "nc" not in _CACHE:
        _CACHE["nc"] = _build()
    return _CACHE["nc"]


def kernel(hidden_state, output_sequence, emb_out, W_dec, b_dec, gamma, beta, W_voc, b_voc):
    emb_out = np.ascontiguousarray(np.asarray(emb_out, dtype=np.float32))
    seq = np.asarray(output_sequence).astype(np.int32)
    W_dec = np.asarray(W_dec, dtype=np.float64)
    b_dec = np.asarray(b_dec, dtype=np.float64)
    gamma = np.asarray(gamma, dtype=np.float64)
    beta = np.asarray(beta, dtype=np.float64)
    W_voc = np.ascontiguousarray(np.asarray(W_voc, dtype=np.float32))
    b_voc = np.asarray(b_voc, dtype=np.float32)

    A = np.eye(E, dtype=np.float64) + W_dec
    Ag = gamma[:, None] * A
    Atg = Ag - np.broadcast_to(Ag.sum(0, keepdims=True) / E, (E, E))
    Abat = np.sqrt(float(E)) * A
    cv = beta @ A + b_dec
    c0 = b_dec - cv

    atg = np.ascontiguousarray(Atg, dtype=np.float32)
    abat = np.ascontiguousarray(Abat, dtype=np.float32)
    cvec = cv.astype(np.float32).reshape(1, E)
    corr0 = c0.astype(np.float32).reshape(1, E)
    gam = gamma.astype(np.float32).reshape(1, E)
    bet = beta.astype(np.float32).reshape(1, E)

    # per-core gather indices: local row L = 1024*i + 32*s + b covers
    # global step t = 32*(8*i + c) + s
    L = np.arange(ROWS)
    blk, s32, b_ = L // 1024, (L % 1024) // 32, L % 32
    in_maps = []
    for c in range(NCORES):
        t = 32 * (NCORES * blk + c) + s32
        gvals = seq[b_, t].astype(np.int32)
        gidx = np.ascontiguousarray(gvals.reshape(NTILES, 128).T)
        in_maps.append({
            "emb": emb_out,
            "gidx": gidx,
            "atg": atg,
            "abat": abat,
            "cvec": cvec,
            "corr0": corr0,
            "gam": gam,
            "bet": bet,
            "wvoc": np.ascontiguousarray(W_voc[:, c * VS : (c + 1) * VS]),
            "bvoc": np.ascontiguousarray(b_voc[c * VS : (c + 1) * VS]).reshape(1, VS),
        })

    nc = _get_nc()
    _CACHE["last_in_maps"] = in_maps
    res = run_bass_kernel_spmd(nc, in_maps, core_ids=list(range(NCORES)))
    z = np.asarray(res.results[0]["z_out"]).reshape(B, 1, E)
    y = np.concatenate(
        [np.asarray(res.results[c]["y_out"]) for c in range(NCORES)], axis=1
    ).reshape(B, 1, V)
    return z, y
